# revision 37
# baseline (speedup 1.0000x reference)
"""Trainium2 Bass kernel for nn_Encoder (KAN-style piecewise-linear MLP encoder).

Math: each adaptive piecewise-linear layer (P=3 knots on [-1,1]) collapses to
    out = u @ A + v @ C + bias,   u = clip(x,-1,1), v = clip(x,0,1).
Since u = v + w with w = clip(x,-1,0) (inputs) or, after an ELU'd hidden
layer, u' = v' + m with m = min(exp(h),1) = exp(min(h,0)), every layer is
re-based on host to the (v, w) feature pair with weights (A+C, A):
    out = v @ (A+C) + w @ A + bias'                (bias' -= colsum(A) where
                                                    the u-feature carried +1)
so no elementwise add is ever needed on chip, and m comes straight out of the
scalar engine's Exp.

LayerNorm mean is free: h - mean_o(h) = x @ (W - rowmean_o(W)) + (b - mean b),
so W1/W2 (and their biases) are column-centered on the host and the kernel only
computes the mean-square stat: var = mean_o(h~^2), rstd = exp(-0.5*ln(var+eps))
on the scalar engine (ln and exp share one activation-table set with
identity/square/copy -> a single table load, no churn).

Sharding: pure data-parallel, batch 16384 -> 8 x 2048. Activations are kept
feature-major ([feat, batch]) on chip; matmul operands are fp16 (full PE rate,
2x/4x DVE modes, half the DMA bytes); PSUM accumulates fp32. The mean-square
stat is a ones-matmul on the PE; rstd's row is broadcast across partitions by
a K=1 ones-matmul. The normalize multiply fuses with the elu clips as DVE
scalar_tensor_tensor ops: ym = min(h,0)*s -> m = Exp(ym);  v' = min(relu(h)*s,
1). gpsimd runs nothing but the input-DMA queue (its ALU measures ~2.3x the
cost model on HW). The za layer's bias rides as a K=9 ones-row inside its own
matmul so its PSUM pair evacuates with a single unbiased copy.

Tile executes each engine's stream in program order, so the batch chunks are
software-pipelined by emission order: the stage sequence interleaves chunk
b+1's matmul stages between chunk b's stats chain and its consumer, keeping
the PE busy through every LayerNorm latency chain. The last layer is computed
batch-major (activations stationary) so the output DMAs from SBUF in the
required [batch, 512] layout (fp16; host casts to fp32).

n_reps>1 wraps the whole computation in a hardware For-loop; used only by the
local timing harness to measure per-iteration device time by wall-clock slope.
"""

import contextlib
import sys

sys.path.insert(0, "/opt/trn_rl_repo")

import numpy as np

import concourse.bass as bass  # noqa: E402
import concourse.tile as tile  # noqa: E402
from concourse import bacc, mybir  # noqa: E402
from concourse.bass_utils import run_bass_kernel_spmd  # noqa: E402

F32 = mybir.dt.float32
F16 = mybir.dt.float16
AF = mybir.ActivationFunctionType
OP = mybir.AluOpType

NCORES = 8
B_LOC = 2048          # batch rows per core
BC = 512              # batch columns per chunk (psum free dim)
NB = B_LOC // BC      # 4 batch chunks
P = 128
LN_EPS = 1e-5


def build_module(n_reps=1, _no_dma=False, _no_in=False, _no_out=False):
    if _no_dma:
        _no_in = _no_out = True
    nc = bacc.Bacc("TRN2", target_bir_lowering=False, debug=False,
                   enable_asserts=False, num_devices=NCORES)

    def din(name, shape, dt=F16):
        return nc.dram_tensor(name, list(shape), dt, kind="ExternalInput").ap()

    # zs feature-major [512, B_LOC]; per-block strided loads measure FASTER
    # on HW than host-bocked fully-contiguous variants (tried 16x128KB
    # contiguous and 4x512KB linear: both ~20us/iter slower).
    zsT = din("zsT", (512, B_LOC))
    actT = din("actT", (8, B_LOC))
    wza = din("wza", (9, 2, 256))     # [0:8,0]=A+C, [8,0]=bias, [0:8,1]=A
    w1 = din("w1", (1536, 512))
    w2 = din("w2", (1024, 512))
    w3 = din("w3", (1024, 512))
    b1_p = din("b1_p", (128, 4), F32)
    b2_p = din("b2_p", (128, 4), F32)
    b3_bc = din("b3_bc", (128, 512), F32)
    ones_c = din("ones_c", (1, 128))
    oinv_m = din("oinv_m", (128, 1))
    out = nc.dram_tensor("out", [B_LOC, 512], F16, kind="ExternalOutput").ap()

    with tile.TileContext(nc) as tc:
        with (
            tc.tile_pool(name="wpool", bufs=1) as wp,
            tc.tile_pool(name="inp", bufs=1) as inp,
            tc.tile_pool(name="work", bufs=2) as wk,
            tc.tile_pool(name="psz", bufs=3, space="PSUM") as psz,
            tc.tile_pool(name="psza", bufs=1, space="PSUM") as psza,
            tc.tile_pool(name="pssq", bufs=2, space="PSUM") as pssq,
            tc.tile_pool(name="psbs", bufs=1, space="PSUM") as psbs,
        ):
            # ---- persistent weights / constants ----
            w1_sb = wp.tile([P, 12, 512], F16)
            nc.sync.dma_start(w1_sb[:], w1.rearrange("(c p) o -> p c o", p=P))
            w2_sb = wp.tile([P, 8, 512], F16)
            nc.sync.dma_start(w2_sb[:], w2.rearrange("(c p) o -> p c o", p=P))
            w3_sb = wp.tile([P, 8, 512], F16)
            nc.sync.dma_start(w3_sb[:], w3.rearrange("(c p) o -> p c o", p=P))
            wza_sb = wp.tile([9, 2, 256], F16)
            nc.sync.dma_start(wza_sb[:], wza[:, :, :])
            b1_sb = wp.tile([P, 4], F32)
            nc.sync.dma_start(b1_sb[:], b1_p[:, :])
            b2_sb = wp.tile([P, 4], F32)
            nc.sync.dma_start(b2_sb[:], b2_p[:, :])
            b3_sb = wp.tile([P, 512], F32)
            nc.sync.dma_start(b3_sb[:], b3_bc[:, :])
            ones_col = wp.tile([1, 128], F16)
            nc.sync.dma_start(ones_col[:], ones_c[:, :])
            oinv_col = wp.tile([P, 1], F16)   # 1/512 -> stats matmul -> mean
            nc.sync.dma_start(oinv_col[:], oinv_m[:, :])
            eps_sb = wp.tile([1, 1], F32)
            nc.vector.memset(eps_sb[:], LN_EPS)

            # Preload the one activation-table set that serves every ACT
            # func used here (Ln, Exp, Identity, Copy) so the compiler's
            # greedy per-func chooser doesn't alternate between the exp-only
            # and ln-only sets (1.3us per reload).
            from concourse.hw_specs import get_activation_tables
            need = {AF.Ln, AF.Exp, AF.Identity, AF.Copy}
            set_id = next(i for i, (_, s) in
                          enumerate(get_activation_tables(nc.m.arch).items())
                          if need <= s)
            nc.scalar.add_instruction(mybir.InstLoadActFuncSet(
                name=nc.get_next_instruction_name(), ins=[], outs=[],
                act_func_set_id=set_id))

            # va_act row 8 is the constant ones-row carrying the za bias;
            # inp pool has bufs=1 so the address is loop-stable and the row
            # only needs initializing once.
            act_raw = inp.tile([8, NB, BC], F16, tag="act_raw")
            va_act = inp.tile([9, NB, BC], F16, tag="va_act")
            wa_act = inp.tile([8, NB, BC], F16, tag="wa_act")
            # memset whole tile: rows 0-7 are rewritten by the clip every
            # iteration; row 8 (start partition 8 is not directly
            # addressable) keeps the 1.0 fill forever.
            nc.vector.memset(va_act[:], 1.0)
            if _no_in:           # timing-ablation builds only
                nc.vector.memset(act_raw[:], 0.25)

            m_za = inp.tile([P, NB, 2, BC], F16, tag="m_za")
            v_za = inp.tile([P, NB, 2, BC], F16, tag="v_za")
            va_all = inp.tile([P, 16, BC], F16, tag="va_all")
            wa_all = inp.tile([P, 16, BC], F16, tag="wa_all")
            if _no_in:           # timing-ablation builds only
                nc.vector.memset(wa_all[:], 0.25)

            def stageZ(b):
                zps2 = psza.tile([P, 2, BC], F32, tag="zza")
                for o in range(2):
                    nc.tensor.matmul(zps2[:, o, :],
                                     wza_sb[:, 0, bass.ts(o, 128)],
                                     va_act[:, b, :],
                                     start=True, stop=False)
                    nc.tensor.matmul(zps2[:, o, :],
                                     wza_sb[0:8, 1, bass.ts(o, 128)],
                                     wa_act[:, b, :],
                                     start=False, stop=True)
                zb = wk.tile([P, 2, BC], F16, tag="zb_za")
                nc.scalar.copy(zb[:], zps2[:])
                ym = wk.tile([P, 2, BC], F16, tag="ym_za")
                nc.vector.tensor_scalar(ym[:], zb[:], 0.0, None, OP.min)
                nc.vector.tensor_scalar(v_za[:, b], zb[:], 0.0, 1.0,
                                        OP.max, OP.min)
                nc.scalar.activation(m_za[:, b], ym[:], AF.Exp)

            def load_zs(b):
                bs = slice(b * 4, b * 4 + 4)
                if not _no_in:
                    for c in range(4):
                        nc.scalar.dma_start(
                            wa_all[:, b * 4 + c, :],
                            zsT[bass.ts(c, 128), bass.ts(b, BC)])
                nc.vector.tensor_scalar(va_all[:, bs], wa_all[:, bs],
                                        0.0, 1.0, OP.max, OP.min)
                nc.vector.tensor_scalar(wa_all[:, bs], wa_all[:, bs],
                                        0.0, -1.0, OP.min, OP.max)

            def prep_za():
                """Load + clip the action and run the za layer -- emitted at
                the TAIL of the body so the next loop iteration (or the
                single real pass, primed by the prologue) starts straight
                into stageA with everything ready.  The big zs reloads are
                NOT here: each chunk reloads right after its last reader
                (stageA(b)), spreading input DMA over the first ~60% of the
                iteration, complementary to the output stores in the tail."""
                if not _no_in:
                    nc.scalar.dma_start(
                        act_raw[:], actT.rearrange("p (b j) -> p b j", b=NB))
                nc.vector.tensor_scalar(va_act[0:8], act_raw[:],
                                        0.0, 1.0, OP.max, OP.min)
                nc.vector.tensor_scalar(wa_act[:], act_raw[:],
                                        0.0, -1.0, OP.min, OP.max)
                for b in range(NB):
                    stageZ(b)

            def prep():
                prep_za()
                for b in range(NB):
                    load_zs(b)

            def body():
                # ---- stage definitions ----
                # mm stage: z-groups + bias + zsq + mean-square stat matmuls,
                # then the norm's cheap row chain (ln -> exp) right at the
                # stage end so rstd is ready long before anyone needs it.
                def stage_mm(KC, w_sb, b_sb, rhs_fn):
                    zcb = wk.tile([P, 4, BC], F16, tag="zcb", bufs=3,
                                  name="zcb")
                    sq_ps = pssq.tile([1, BC], F32, tag="sq")
                    for o in range(4):
                        zps = psz.tile([P, BC], F32, tag="z")
                        for k in range(KC):
                            nc.tensor.matmul(zps[:],
                                             w_sb[:, k, bass.ts(o, 128)],
                                             rhs_fn(k),
                                             start=(k == 0),
                                             stop=(k == KC - 1))
                        nc.scalar.activation(zcb[:, o, :], zps[:],
                                             AF.Identity,
                                             bias=b_sb[:, o:o + 1])
                        zsq = wk.tile([P, BC], F16, tag="zsq")
                        nc.vector.tensor_mul(zsq[:], zcb[:, o, :],
                                             zcb[:, o, :])
                        nc.tensor.matmul(sq_ps[:], oinv_col[:], zsq[:],
                                         start=(o == 0), stop=(o == 3))
                    lnv = wk.tile([1, BC], F32, tag="lnv")
                    nc.scalar.activation(lnv[:], sq_ps[:], AF.Ln,
                                         bias=eps_sb[:])
                    rstd = wk.tile([1, BC], F16, tag="rstd", bufs=3,
                                   name="rstd")
                    nc.scalar.activation(rstd[:], lnv[:], AF.Exp, scale=-0.5)
                    return zcb, rstd

                # norm bulk, emitted one mm-stage after its producer so every
                # input (zcb, rstd) is long ready and no engine queue stalls:
                # broadcast rstd across partitions via K=1 ones-matmul, then
                #   y  = h*s (in place);  v' = clip(y,0,1)
                #   m  = Exp(min(y,0))    (= min(exp(y),1))
                def norm_rest(zcb, rstd, m_dst, v_dst):
                    sb_ps = psbs.tile([P, BC], F32, tag="sb")
                    nc.tensor.matmul(sb_ps[:], ones_col[:], rstd[:],
                                     start=True, stop=True)
                    bcb = wk.tile([P, 1, BC], F16, tag="bcb")
                    nc.vector.tensor_copy(bcb[:, 0, :], sb_ps[:])
                    bce = bcb[:, 0:1, :].to_broadcast([P, 4, BC])
                    nc.vector.tensor_mul(zcb[:], zcb[:], bce)   # y = h*s
                    nc.vector.tensor_scalar(v_dst[:], zcb[:], 0.0, 1.0,
                                            OP.max, OP.min)
                    ym = wk.tile([P, 4, BC], F16, tag="ym")
                    nc.vector.tensor_scalar(ym[:], zcb[:], 0.0, None, OP.min)
                    nc.scalar.activation(m_dst[:], ym[:], AF.Exp)

                # ---- software pipeline over the NB=4 batch chunks ----
                st1 = [None] * NB   # (zcb, rstd) from l1 mm stage
                st2 = [None] * NB
                m1 = [None] * NB
                v1 = [None] * NB
                m2 = [None] * NB
                v2 = [None] * NB

                def stageA(b):
                    def rhs1(k):
                        if k < 4:
                            return va_all[:, b * 4 + k, :]
                        if k < 8:
                            return wa_all[:, b * 4 + (k - 4), :]
                        if k < 10:
                            return v_za[:, b, k - 8, :]
                        return m_za[:, b, k - 10, :]
                    st1[b] = stage_mm(12, w1_sb, b1_sb, rhs1)

                def rest_n(b):
                    m1[b] = wk.tile([P, 4, BC], F16, tag="m1", bufs=3,
                                    name="m1")
                    v1[b] = wk.tile([P, 4, BC], F16, tag="v1", bufs=3,
                                    name="v1")
                    norm_rest(*st1[b], m1[b], v1[b])

                def stageB(b):
                    def rhs2(k):
                        return v1[b][:, k, :] if k < 4 else m1[b][:, k - 4, :]
                    st2[b] = stage_mm(8, w2_sb, b2_sb, rhs2)

                def rest_m(b):
                    m2[b] = wk.tile([P, 4, BC], F16, tag="m2", bufs=3,
                                    name="m2")
                    v2[b] = wk.tile([P, 4, BC], F16, tag="v2", bufs=3,
                                    name="v2")
                    norm_rest(*st2[b], m2[b], v2[b])

                def stageC(b):
                    for q in range(4):
                        qs = bass.ts(q, 128)
                        ops = psz.tile([P, 512], F32, tag="z")
                        for k in range(8):
                            lhsT = (v2[b][:, k, qs] if k < 4
                                    else m2[b][:, k - 4, qs])
                            nc.tensor.matmul(ops[:], lhsT, w3_sb[:, k, :],
                                             start=(k == 0), stop=(k == 7))
                        osb = wk.tile([P, 512], F16, tag="osb", bufs=4,
                                      name="osb")
                        nc.vector.scalar_tensor_tensor(osb[:], ops[:], 1.0,
                                                       b3_sb[:], OP.mult,
                                                       OP.add)
                        if not _no_out:
                            nc.sync.dma_start(out[b * BC + q * 128:
                                                  b * BC + (q + 1) * 128, :],
                                              osb[:])

                # emission order IS the per-engine execution order.  Each
                # norm's bulk block (rest_*) sits between two mm stages, one
                # full stage after its producer and at least one stage before
                # its consumer, so every dependency has ~a-full-mm-stage of
                # slack and the PE never waits on a norm chain.  Inputs for
                # this pass were prepared by the previous pass's prep() (or
                # the prologue), so stageA(0) starts immediately.
                stageA(0)
                load_zs(0)      # next pass's chunk-0 (WAR just released)
                stageA(1)
                load_zs(1)
                rest_n(0)
                stageA(2)
                load_zs(2)
                rest_n(1)
                stageB(0)
                rest_n(2)
                stageA(3)
                load_zs(3)
                rest_m(0)
                stageB(1)
                rest_n(3)
                stageB(2)
                rest_m(1)
                stageC(0)
                rest_m(2)
                stageB(3)
                stageC(1)
                rest_m(3)
                stageC(2)
                stageC(3)
                prep_za()   # action + za for the NEXT pass

            prep()          # prologue: prime the first pass
            rep_ctx = (tc.For_i(0, n_reps, 1) if n_reps > 1
                       else contextlib.nullcontext())
            with rep_ctx:
                body()

    nc.compile()
    return nc


def fold_weights(W_za, W1, W2, W3):
    def fold(vals):
        V = vals.astype(np.float64)
        A = V[:, :, 1] - V[:, :, 0]
        C = V[:, :, 0] + V[:, :, 2] - 2.0 * V[:, :, 1]
        b = V[:, :, 1].sum(axis=0)
        return A, C, b

    A0, C0, b0 = fold(W_za)
    A1, C1, b1 = fold(W1)
    A2, C2, b2 = fold(W2)
    A3, C3, b3 = fold(W3)

    # (v, w) basis: v-features weight (A+C), w-features weight A.
    wza = np.zeros((9, 2, 256))
    wza[0:8, 0] = A0 + C0
    wza[8, 0] = b0                       # ones-row carries the za bias
    wza[0:8, 1] = A0

    S1 = A1 + C1
    w1 = np.concatenate([S1[:512], A1[:512], S1[512:], A1[512:]], axis=0)
    w2 = np.concatenate([A2 + C2, A2], axis=0)           # [1024, 512]
    w3 = np.concatenate([A3 + C3, A3], axis=0)           # [1024, 512]
    b1e = b1 - A1[512:].sum(axis=0)      # za's u = v + m - 1
    b2e = b2 - A2.sum(axis=0)
    b3e = b3 - A3.sum(axis=0)

    # LayerNorm mean subtraction folded into the weights: h - mean_o(h) =
    # x @ (W - rowmean(W)) + (b - mean(b)).  Applies to the two LN'd layers.
    w1 = w1 - w1.mean(axis=1, keepdims=True)
    b1e = b1e - b1e.mean()
    w2 = w2 - w2.mean(axis=1, keepdims=True)
    b2e = b2e - b2e.mean()

    f, h = np.float32, np.float16
    return {
        "wza": np.ascontiguousarray(wza, h),
        "w1": np.ascontiguousarray(w1, h),
        "w2": np.ascontiguousarray(w2, h),
        "w3": np.ascontiguousarray(w3, h),
        "b1_p": np.ascontiguousarray(b1e.reshape(4, 128).T, f),
        "b2_p": np.ascontiguousarray(b2e.reshape(4, 128).T, f),
        "b3_bc": np.ascontiguousarray(np.broadcast_to(b3e, (128, 512)), f),
        "ones_c": np.ones((1, 128), h),
        "oinv_m": np.full((128, 1), 1.0 / 512.0, h),
    }


_NC_CACHE = {}


def get_module(n_reps=1):
    key = f"nc{n_reps}"
    if key not in _NC_CACHE:
        _NC_CACHE[key] = build_module(n_reps)
    return _NC_CACHE[key]


def make_in_maps(zs, action, W_za, W1, W2, W3):
    wmap = fold_weights(np.asarray(W_za), np.asarray(W1), np.asarray(W2),
                        np.asarray(W3))
    in_maps = []
    for c in range(NCORES):
        sl = slice(c * B_LOC, (c + 1) * B_LOC)
        m = dict(wmap)
        m["zsT"] = np.ascontiguousarray(np.asarray(zs)[sl].T, np.float16)
        m["actT"] = np.ascontiguousarray(np.asarray(action)[sl].T, np.float16)
        in_maps.append(m)
    return in_maps


def kernel(zs, action, W_za, W1, W2, W3, _trace=False, _tmpdir=None):
    nc = get_module()
    in_maps = make_in_maps(zs, action, W_za, W1, W2, W3)
    res = run_bass_kernel_spmd(nc, in_maps, core_ids=list(range(NCORES)),
                               trace=_trace, tmpdir=_tmpdir)
    out = np.concatenate([res.results[c]["out"] for c in range(NCORES)],
                         axis=0).astype(np.float32)
    if _trace:
        kernel.last_exec_time_ns = res.exec_time_ns
        kernel.last_results = res
    return out


# revision 39
# speedup vs baseline: 1.1152x; 1.1152x over previous
"""Trainium2 Bass kernel for nn_Encoder (KAN-style piecewise-linear MLP encoder).

Math: each adaptive piecewise-linear layer (P=3 knots on [-1,1]) collapses to
    out = u @ A + v @ C + bias,   u = clip(x,-1,1), v = clip(x,0,1).
Since u = v + w with w = clip(x,-1,0) (inputs) or, after an ELU'd hidden
layer, u' = v' + m with m = min(exp(h),1) = exp(min(h,0)), every layer is
re-based on host to the (v, w) feature pair with weights (A+C, A):
    out = v @ (A+C) + w @ A + bias'                (bias' -= colsum(A) where
                                                    the u-feature carried +1)
so no elementwise add is ever needed on chip, and m comes straight out of the
scalar engine's Exp.

LayerNorm mean is free: h - mean_o(h) = x @ (W - rowmean_o(W)) + (b - mean b),
so W1/W2 (and their biases) are column-centered on the host and the kernel only
computes the mean-square stat: var = mean_o(h~^2), rstd = exp(-0.5*ln(var+eps))
on the scalar engine (ln and exp share one activation-table set with
identity/square/copy -> a single table load, no churn).

Sharding: pure data-parallel, batch 16384 -> 8 x 2048. Activations are kept
feature-major ([feat, batch]) on chip; matmul operands are fp16 (full PE rate,
2x/4x DVE modes, half the DMA bytes); PSUM accumulates fp32. The mean-square
stat is a ones-matmul on the PE; rstd's row is broadcast across partitions by
a K=1 ones-matmul. The normalize multiply fuses with the elu clips as DVE
scalar_tensor_tensor ops: ym = min(h,0)*s -> m = Exp(ym);  v' = min(relu(h)*s,
1). gpsimd runs nothing but the input-DMA queue (its ALU measures ~2.3x the
cost model on HW). The za layer's bias rides as a K=9 ones-row inside its own
matmul so its PSUM pair evacuates with a single unbiased copy.

Tile executes each engine's stream in program order, so the batch chunks are
software-pipelined by emission order: the stage sequence interleaves chunk
b+1's matmul stages between chunk b's stats chain and its consumer, keeping
the PE busy through every LayerNorm latency chain. The last layer is computed
batch-major (activations stationary) so the output DMAs from SBUF in the
required [batch, 512] layout (fp16; host casts to fp32).

n_reps>1 wraps the whole computation in a hardware For-loop; used only by the
local timing harness to measure per-iteration device time by wall-clock slope.
"""

import contextlib
import sys

sys.path.insert(0, "/opt/trn_rl_repo")

import numpy as np

import concourse.bass as bass  # noqa: E402
import concourse.tile as tile  # noqa: E402
from concourse import bacc, mybir  # noqa: E402
from concourse.bass_utils import run_bass_kernel_spmd  # noqa: E402

F32 = mybir.dt.float32
F16 = mybir.dt.float16
AF = mybir.ActivationFunctionType
OP = mybir.AluOpType

NCORES = 8
B_LOC = 2048          # batch rows per core
BC = 512              # batch columns per chunk (psum free dim)
NB = B_LOC // BC      # 4 batch chunks
P = 128
LN_EPS = 1e-5


def build_module(n_reps=1, _no_dma=False, _no_in=False, _no_out=False):
    if _no_dma:
        _no_in = _no_out = True
    nc = bacc.Bacc("TRN2", target_bir_lowering=False, debug=False,
                   enable_asserts=False, num_devices=NCORES)

    def din(name, shape, dt=F16):
        return nc.dram_tensor(name, list(shape), dt, kind="ExternalInput").ap()

    # zs feature-major [512, B_LOC]; per-block strided loads measure FASTER
    # on HW than host-bocked fully-contiguous variants (tried 16x128KB
    # contiguous and 4x512KB linear: both ~20us/iter slower).
    zsT = din("zsT", (512, B_LOC))
    actT = din("actT", (8, B_LOC))
    wza = din("wza", (9, 2, 256))     # [0:8,0]=A+C, [8,0]=bias, [0:8,1]=A
    w1 = din("w1", (1536, 512))
    w2 = din("w2", (1024, 512))
    w3 = din("w3", (1024, 512))
    b1_p = din("b1_p", (128, 4), F32)
    b2_p = din("b2_p", (128, 4), F32)
    b3_bc = din("b3_bc", (128, 512), F32)
    ones_c = din("ones_c", (1, 128))
    oinv_m = din("oinv_m", (128, 1))
    out = nc.dram_tensor("out", [B_LOC, 512], F16, kind="ExternalOutput").ap()

    with tile.TileContext(nc) as tc:
        with (
            tc.tile_pool(name="wpool", bufs=1) as wp,
            tc.tile_pool(name="inp", bufs=1) as inp,
            tc.tile_pool(name="work", bufs=2) as wk,
            tc.tile_pool(name="psz", bufs=3, space="PSUM") as psz,
            tc.tile_pool(name="psza", bufs=1, space="PSUM") as psza,
            tc.tile_pool(name="pssq", bufs=2, space="PSUM") as pssq,
            tc.tile_pool(name="psbs", bufs=1, space="PSUM") as psbs,
        ):
            # ---- persistent weights / constants ----
            w1_sb = wp.tile([P, 12, 512], F16)
            nc.sync.dma_start(w1_sb[:], w1.rearrange("(c p) o -> p c o", p=P))
            w2_sb = wp.tile([P, 8, 512], F16)
            nc.sync.dma_start(w2_sb[:], w2.rearrange("(c p) o -> p c o", p=P))
            w3_sb = wp.tile([P, 8, 512], F16)
            nc.sync.dma_start(w3_sb[:], w3.rearrange("(c p) o -> p c o", p=P))
            wza_sb = wp.tile([9, 2, 256], F16)
            nc.sync.dma_start(wza_sb[:], wza[:, :, :])
            b1_sb = wp.tile([P, 4], F32)
            nc.sync.dma_start(b1_sb[:], b1_p[:, :])
            b2_sb = wp.tile([P, 4], F32)
            nc.sync.dma_start(b2_sb[:], b2_p[:, :])
            b3_sb = wp.tile([P, 512], F32)
            nc.sync.dma_start(b3_sb[:], b3_bc[:, :])
            ones_col = wp.tile([1, 128], F16)
            nc.sync.dma_start(ones_col[:], ones_c[:, :])
            oinv_col = wp.tile([P, 1], F16)   # 1/512 -> stats matmul -> mean
            nc.sync.dma_start(oinv_col[:], oinv_m[:, :])
            eps_sb = wp.tile([1, 1], F32)
            nc.vector.memset(eps_sb[:], LN_EPS)

            # Preload the one activation-table set that serves every ACT
            # func used here (Ln, Exp, Identity, Copy) so the compiler's
            # greedy per-func chooser doesn't alternate between the exp-only
            # and ln-only sets (1.3us per reload).
            from concourse.hw_specs import get_activation_tables
            need = {AF.Ln, AF.Exp, AF.Identity, AF.Copy}
            set_id = next(i for i, (_, s) in
                          enumerate(get_activation_tables(nc.m.arch).items())
                          if need <= s)
            nc.scalar.add_instruction(mybir.InstLoadActFuncSet(
                name=nc.get_next_instruction_name(), ins=[], outs=[],
                act_func_set_id=set_id))

            # va_act row 8 is the constant ones-row carrying the za bias;
            # inp pool has bufs=1 so the address is loop-stable and the row
            # only needs initializing once.
            act_raw = inp.tile([8, NB, BC], F16, tag="act_raw")
            va_act = inp.tile([9, NB, BC], F16, tag="va_act")
            wa_act = inp.tile([8, NB, BC], F16, tag="wa_act")
            # memset whole tile: rows 0-7 are rewritten by the clip every
            # iteration; row 8 (start partition 8 is not directly
            # addressable) keeps the 1.0 fill forever.
            nc.vector.memset(va_act[:], 1.0)
            if _no_in:           # timing-ablation builds only
                nc.vector.memset(act_raw[:], 0.25)

            m_za = inp.tile([P, NB, 2, BC], F16, tag="m_za")
            v_za = inp.tile([P, NB, 2, BC], F16, tag="v_za")
            va_all = inp.tile([P, 16, BC], F16, tag="va_all")
            wa_all = inp.tile([P, 16, BC], F16, tag="wa_all")
            if _no_in:           # timing-ablation builds only
                nc.vector.memset(wa_all[:], 0.25)

            def stageZ(b):
                zps2 = psza.tile([P, 2, BC], F32, tag="zza")
                for o in range(2):
                    nc.tensor.matmul(zps2[:, o, :],
                                     wza_sb[:, 0, bass.ts(o, 128)],
                                     va_act[:, b, :],
                                     start=True, stop=False)
                    nc.tensor.matmul(zps2[:, o, :],
                                     wza_sb[0:8, 1, bass.ts(o, 128)],
                                     wa_act[:, b, :],
                                     start=False, stop=True)
                zb = wk.tile([P, 2, BC], F16, tag="zb_za")
                nc.scalar.copy(zb[:], zps2[:])
                ym = wk.tile([P, 2, BC], F16, tag="ym_za")
                nc.vector.tensor_scalar(ym[:], zb[:], 0.0, None, OP.min)
                nc.vector.tensor_scalar(v_za[:, b], zb[:], 0.0, 1.0,
                                        OP.max, OP.min)
                nc.scalar.activation(m_za[:, b], ym[:], AF.Exp)

            def load_zs(b):
                bs = slice(b * 4, b * 4 + 4)
                if not _no_in:
                    for c in range(4):
                        nc.gpsimd.dma_start(
                            wa_all[:, b * 4 + c, :],
                            zsT[bass.ts(c, 128), bass.ts(b, BC)])
                nc.vector.tensor_scalar(va_all[:, bs], wa_all[:, bs],
                                        0.0, 1.0, OP.max, OP.min)
                nc.vector.tensor_scalar(wa_all[:, bs], wa_all[:, bs],
                                        0.0, -1.0, OP.min, OP.max)

            def prep_za():
                """Load + clip the action and run the za layer -- emitted at
                the TAIL of the body so the next loop iteration (or the
                single real pass, primed by the prologue) starts straight
                into stageA with everything ready.  The big zs reloads are
                NOT here: each chunk reloads right after its last reader
                (stageA(b)), spreading input DMA over the first ~60% of the
                iteration, complementary to the output stores in the tail."""
                if not _no_in:
                    nc.gpsimd.dma_start(
                        act_raw[:], actT.rearrange("p (b j) -> p b j", b=NB))
                nc.vector.tensor_scalar(va_act[0:8], act_raw[:],
                                        0.0, 1.0, OP.max, OP.min)
                nc.vector.tensor_scalar(wa_act[:], act_raw[:],
                                        0.0, -1.0, OP.min, OP.max)
                for b in range(NB):
                    stageZ(b)

            def prep():
                prep_za()
                for b in range(NB):
                    load_zs(b)

            def body():
                # ---- stage definitions ----
                # mm stage: z-groups + bias + zsq + mean-square stat matmuls,
                # then the norm's cheap row chain (ln -> exp) right at the
                # stage end so rstd is ready long before anyone needs it.
                def stage_mm(KC, w_sb, b_sb, rhs_fn):
                    zcb = wk.tile([P, 4, BC], F16, tag="zcb", bufs=3,
                                  name="zcb")
                    sq_ps = pssq.tile([1, BC], F32, tag="sq")
                    for o in range(4):
                        zps = psz.tile([P, BC], F32, tag="z")
                        for k in range(KC):
                            nc.tensor.matmul(zps[:],
                                             w_sb[:, k, bass.ts(o, 128)],
                                             rhs_fn(k),
                                             start=(k == 0),
                                             stop=(k == KC - 1))
                        nc.scalar.activation(zcb[:, o, :], zps[:],
                                             AF.Identity,
                                             bias=b_sb[:, o:o + 1])
                        zsq = wk.tile([P, BC], F16, tag="zsq")
                        nc.vector.tensor_mul(zsq[:], zcb[:, o, :],
                                             zcb[:, o, :])
                        nc.tensor.matmul(sq_ps[:], oinv_col[:], zsq[:],
                                         start=(o == 0), stop=(o == 3))
                    lnv = wk.tile([1, BC], F32, tag="lnv")
                    nc.scalar.activation(lnv[:], sq_ps[:], AF.Ln,
                                         bias=eps_sb[:])
                    rstd = wk.tile([1, BC], F16, tag="rstd", bufs=3,
                                   name="rstd")
                    nc.scalar.activation(rstd[:], lnv[:], AF.Exp, scale=-0.5)
                    return zcb, rstd

                # norm bulk, emitted one mm-stage after its producer so every
                # input (zcb, rstd) is long ready and no engine queue stalls:
                # broadcast rstd across partitions via K=1 ones-matmul, then
                #   y  = h*s (in place);  v' = clip(y,0,1)
                #   m  = Exp(min(y,0))    (= min(exp(y),1))
                def norm_rest(zcb, rstd, m_dst, v_dst):
                    sb_ps = psbs.tile([P, BC], F32, tag="sb")
                    nc.tensor.matmul(sb_ps[:], ones_col[:], rstd[:],
                                     start=True, stop=True)
                    bcb = wk.tile([P, 1, BC], F16, tag="bcb")
                    nc.vector.tensor_copy(bcb[:, 0, :], sb_ps[:])
                    bce = bcb[:, 0:1, :].to_broadcast([P, 4, BC])
                    nc.vector.tensor_mul(zcb[:], zcb[:], bce)   # y = h*s
                    nc.vector.tensor_scalar(v_dst[:], zcb[:], 0.0, 1.0,
                                            OP.max, OP.min)
                    ym = wk.tile([P, 4, BC], F16, tag="ym")
                    nc.vector.tensor_scalar(ym[:], zcb[:], 0.0, None, OP.min)
                    nc.scalar.activation(m_dst[:], ym[:], AF.Exp)

                # ---- software pipeline over the NB=4 batch chunks ----
                st1 = [None] * NB   # (zcb, rstd) from l1 mm stage
                st2 = [None] * NB
                m1 = [None] * NB
                v1 = [None] * NB
                m2 = [None] * NB
                v2 = [None] * NB

                def stageA(b):
                    def rhs1(k):
                        if k < 4:
                            return va_all[:, b * 4 + k, :]
                        if k < 8:
                            return wa_all[:, b * 4 + (k - 4), :]
                        if k < 10:
                            return v_za[:, b, k - 8, :]
                        return m_za[:, b, k - 10, :]
                    st1[b] = stage_mm(12, w1_sb, b1_sb, rhs1)

                def rest_n(b):
                    m1[b] = wk.tile([P, 4, BC], F16, tag="m1", bufs=3,
                                    name="m1")
                    v1[b] = wk.tile([P, 4, BC], F16, tag="v1", bufs=3,
                                    name="v1")
                    norm_rest(*st1[b], m1[b], v1[b])

                def stageB(b):
                    def rhs2(k):
                        return v1[b][:, k, :] if k < 4 else m1[b][:, k - 4, :]
                    st2[b] = stage_mm(8, w2_sb, b2_sb, rhs2)

                def rest_m(b):
                    m2[b] = wk.tile([P, 4, BC], F16, tag="m2", bufs=3,
                                    name="m2")
                    v2[b] = wk.tile([P, 4, BC], F16, tag="v2", bufs=3,
                                    name="v2")
                    norm_rest(*st2[b], m2[b], v2[b])

                def stageC(b):
                    for q in range(4):
                        qs = bass.ts(q, 128)
                        ops = psz.tile([P, 512], F32, tag="z")
                        for k in range(8):
                            lhsT = (v2[b][:, k, qs] if k < 4
                                    else m2[b][:, k - 4, qs])
                            nc.tensor.matmul(ops[:], lhsT, w3_sb[:, k, :],
                                             start=(k == 0), stop=(k == 7))
                        osb = wk.tile([P, 512], F16, tag="osb", bufs=4,
                                      name="osb")
                        nc.vector.scalar_tensor_tensor(osb[:], ops[:], 1.0,
                                                       b3_sb[:], OP.mult,
                                                       OP.add)
                        if not _no_out:
                            nc.sync.dma_start(out[b * BC + q * 128:
                                                  b * BC + (q + 1) * 128, :],
                                              osb[:])

                # emission order IS the per-engine execution order.  Each
                # norm's bulk block (rest_*) sits between two mm stages, one
                # full stage after its producer and at least one stage before
                # its consumer, so every dependency has ~a-full-mm-stage of
                # slack and the PE never waits on a norm chain.  Inputs for
                # this pass were prepared by the previous pass's prep() (or
                # the prologue), so stageA(0) starts immediately.
                stageA(0)
                load_zs(0)      # next pass's chunk-0 (WAR just released)
                stageA(1)
                load_zs(1)
                rest_n(0)
                stageA(2)
                load_zs(2)
                rest_n(1)
                stageB(0)
                rest_n(2)
                stageA(3)
                load_zs(3)
                rest_m(0)
                stageB(1)
                rest_n(3)
                stageB(2)
                rest_m(1)
                stageC(0)
                rest_m(2)
                stageB(3)
                stageC(1)
                rest_m(3)
                stageC(2)
                stageC(3)
                prep_za()   # action + za for the NEXT pass

            prep()          # prologue: prime the first pass
            rep_ctx = (tc.For_i(0, n_reps, 1, staggered_reset=True)
                       if n_reps > 1 else contextlib.nullcontext())
            with rep_ctx:
                body()

    nc.compile()
    return nc


def fold_weights(W_za, W1, W2, W3):
    def fold(vals):
        V = vals.astype(np.float64)
        A = V[:, :, 1] - V[:, :, 0]
        C = V[:, :, 0] + V[:, :, 2] - 2.0 * V[:, :, 1]
        b = V[:, :, 1].sum(axis=0)
        return A, C, b

    A0, C0, b0 = fold(W_za)
    A1, C1, b1 = fold(W1)
    A2, C2, b2 = fold(W2)
    A3, C3, b3 = fold(W3)

    # (v, w) basis: v-features weight (A+C), w-features weight A.
    wza = np.zeros((9, 2, 256))
    wza[0:8, 0] = A0 + C0
    wza[8, 0] = b0                       # ones-row carries the za bias
    wza[0:8, 1] = A0

    S1 = A1 + C1
    w1 = np.concatenate([S1[:512], A1[:512], S1[512:], A1[512:]], axis=0)
    w2 = np.concatenate([A2 + C2, A2], axis=0)           # [1024, 512]
    w3 = np.concatenate([A3 + C3, A3], axis=0)           # [1024, 512]
    b1e = b1 - A1[512:].sum(axis=0)      # za's u = v + m - 1
    b2e = b2 - A2.sum(axis=0)
    b3e = b3 - A3.sum(axis=0)

    # LayerNorm mean subtraction folded into the weights: h - mean_o(h) =
    # x @ (W - rowmean(W)) + (b - mean(b)).  Applies to the two LN'd layers.
    w1 = w1 - w1.mean(axis=1, keepdims=True)
    b1e = b1e - b1e.mean()
    w2 = w2 - w2.mean(axis=1, keepdims=True)
    b2e = b2e - b2e.mean()

    f, h = np.float32, np.float16
    return {
        "wza": np.ascontiguousarray(wza, h),
        "w1": np.ascontiguousarray(w1, h),
        "w2": np.ascontiguousarray(w2, h),
        "w3": np.ascontiguousarray(w3, h),
        "b1_p": np.ascontiguousarray(b1e.reshape(4, 128).T, f),
        "b2_p": np.ascontiguousarray(b2e.reshape(4, 128).T, f),
        "b3_bc": np.ascontiguousarray(np.broadcast_to(b3e, (128, 512)), f),
        "ones_c": np.ones((1, 128), h),
        "oinv_m": np.full((128, 1), 1.0 / 512.0, h),
    }


_NC_CACHE = {}


def get_module(n_reps=1):
    key = f"nc{n_reps}"
    if key not in _NC_CACHE:
        _NC_CACHE[key] = build_module(n_reps)
    return _NC_CACHE[key]


def make_in_maps(zs, action, W_za, W1, W2, W3):
    wmap = fold_weights(np.asarray(W_za), np.asarray(W1), np.asarray(W2),
                        np.asarray(W3))
    in_maps = []
    for c in range(NCORES):
        sl = slice(c * B_LOC, (c + 1) * B_LOC)
        m = dict(wmap)
        m["zsT"] = np.ascontiguousarray(np.asarray(zs)[sl].T, np.float16)
        m["actT"] = np.ascontiguousarray(np.asarray(action)[sl].T, np.float16)
        in_maps.append(m)
    return in_maps


def kernel(zs, action, W_za, W1, W2, W3, _trace=False, _tmpdir=None):
    nc = get_module()
    in_maps = make_in_maps(zs, action, W_za, W1, W2, W3)
    res = run_bass_kernel_spmd(nc, in_maps, core_ids=list(range(NCORES)),
                               trace=_trace, tmpdir=_tmpdir)
    out = np.concatenate([res.results[c]["out"] for c in range(NCORES)],
                         axis=0).astype(np.float32)
    if _trace:
        kernel.last_exec_time_ns = res.exec_time_ns
        kernel.last_results = res
    return out
